# revision 3
# baseline (speedup 1.0000x reference)
"""DLDMD kernel for Trainium2 (8 NeuronCores, batch-sharded).

Device (Bass/Tile, SPMD over 8 cores, 64 trajectories each):
  - encoder MLP   x  [B,T,3]  -> y     [B,T,32]   (fp32 matmuls, exact)
  - decoder MLP   y          -> x_ae   [B,T,3]    (float32r matmuls)
  - decoder MLP   y_adv      -> x_adv  [B,P,3]    (float32r matmuls)
Host (jnp on CPU, replicating the reference's EDMD chain op-for-op):
  - SVD -> A -> eig -> phi -> Vandermonde powers -> y_adv
  (complex nonsymmetric eig has no Trainium implementation; the chain is
  numerically chaotic so it must be replicated with the identical LAPACK
  calls the reference uses, seeded by the device-computed y.)
"""

import numpy as np

B, T, P_STEPS = 512, 256, 256
PHYS, LAT, NEUR, NLAYERS = 3, 32, 256, 4
N_CORES = 8
BPC = B // N_CORES              # trajectories per core
TOK_ENC = BPC * T               # encoder tokens per core
TOK_DEC = 2 * TOK_ENC           # decoder tokens per core (y ++ y_adv)
SC = 1024                       # tokens per superchunk (2 psum banks per tile)
MM = 512                        # moving-operand free size per matmul (fp32 max)
P = 128

# weight-pack column layout (one [128, WCOLS] fp32 array, single DMA)
_W_IN0 = 0                       # w_in   rows 0:Fin        cols [0, NEUR)
_W_H0 = NEUR                     # w_h    (i,k) -> NEUR cols each
_W_OUT0 = _W_H0 + NLAYERS * 2 * NEUR
def _pack_cols(fout):
    w_out0 = _W_OUT0
    b_in0 = w_out0 + 2 * fout
    b_h0 = b_in0 + 2
    b_out0 = b_h0 + NLAYERS * 2
    return w_out0, b_in0, b_h0, b_out0, b_out0 + 1


def _pack_weights(w_in, b_in, w_h, b_h, w_out, b_out):
    fin, fout = w_in.shape[0], w_out.shape[1]
    w_out0, b_in0, b_h0, b_out0, wcols = _pack_cols(fout)
    pk = np.zeros((P, wcols), np.float32)
    pk[0:fin, _W_IN0:_W_IN0 + NEUR] = w_in
    for i in range(NLAYERS):
        for k in range(2):
            pk[:, _W_H0 + (i * 2 + k) * NEUR: _W_H0 + (i * 2 + k + 1) * NEUR] = \
                w_h[i, k * P:(k + 1) * P, :]
    for k in range(2):
        pk[:, w_out0 + k * fout: w_out0 + (k + 1) * fout] = w_out[k * P:(k + 1) * P, :]
    pk[:, b_in0] = b_in[0:P]
    pk[:, b_in0 + 1] = b_in[P:NEUR]
    for i in range(NLAYERS):
        for m in range(2):
            pk[:, b_h0 + i * 2 + m] = b_h[i, m * P:(m + 1) * P]
    pk[0:fout, b_out0] = b_out
    return pk


def _build_mlp_module(fin, fout, ntok, use_f32r):
    """One SPMD module: xt [fin, ntok] -> yt [fout, ntok] through the MLP."""
    import concourse.bacc as bacc
    import concourse.tile as tile
    import concourse.mybir as mybir

    F32 = mybir.dt.float32
    MMDT = mybir.dt.float32r if use_f32r else F32
    AFT = mybir.ActivationFunctionType

    w_out0, b_in0, b_h0, b_out0, wcols = _pack_cols(fout)

    nc = bacc.Bacc("TRN2", target_bir_lowering=False, debug=False,
                   num_devices=N_CORES)
    xt_d = nc.dram_tensor("xt", [fin, ntok], MMDT, kind="ExternalInput").ap()
    w_d = nc.dram_tensor("wpack", [P, wcols], MMDT, kind="ExternalInput").ap()
    yt_d = nc.dram_tensor("yt", [fout, ntok], F32, kind="ExternalOutput").ap()

    nsc = ntok // SC
    with tile.TileContext(nc) as tc:
        with tc.tile_pool(name="wp", bufs=1) as wp, \
             tc.tile_pool(name="ap", bufs=3) as apool, \
             tc.tile_pool(name="hp", bufs=6) as hpool, \
             tc.tile_pool(name="op", bufs=3) as opool, \
             tc.tile_pool(name="ps", bufs=3, space="PSUM") as psp, \
             tc.tile_pool(name="pso", bufs=1, space="PSUM") as psop:
            ws = wp.tile([P, wcols], MMDT)
            nc.sync.dma_start(ws[:], w_d[:, :])
            # dummy matmul: absorbs the weight-DMA wait so every real matmul
            # needs at most one sync wait (LDWEIGHTS allows only one).
            dps = psop.tile([1, 1], F32, tag="pso", name="dummy_ps")
            nc.tensor.matmul(dps[0:1, 0:1], ws[:, 0:1].bitcast(F32),
                             ws[:, 1:2].bitcast(F32), start=True, stop=True)

            def bias(col, rows=P):
                return ws[0:rows, col:col + 1].bitcast(F32)

            for c in range(nsc):
                a = apool.tile([fin, SC], MMDT, tag="a")
                nc.sync.dma_start(a[:], xt_d[:, c * SC:(c + 1) * SC])
                # input layer: fin -> NEUR, tanh
                h = [hpool.tile([P, SC], MMDT, tag="h", name=f"h{c}_in{m}") for m in range(2)]
                for m in range(2):
                    ps = psp.tile([P, SC], F32, tag="ps")
                    for j in range(SC // MM):
                        nc.tensor.matmul(
                            ps[:, j * MM:(j + 1) * MM],
                            ws[0:fin, _W_IN0 + m * P: _W_IN0 + (m + 1) * P],
                            a[:, j * MM:(j + 1) * MM], start=True, stop=True)
                    nc.scalar.activation(h[m][:], ps[:], AFT.Tanh,
                                         bias=bias(b_in0 + m))
                # hidden layers: NEUR -> NEUR, tanh
                for i in range(NLAYERS):
                    h2 = [hpool.tile([P, SC], MMDT, tag="h", name=f"h{c}_l{i}_{m}") for m in range(2)]
                    for m in range(2):
                        ps = psp.tile([P, SC], F32, tag="ps")
                        for j in range(SC // MM):
                            for k in range(2):
                                nc.tensor.matmul(
                                    ps[:, j * MM:(j + 1) * MM],
                                    ws[:, _W_H0 + (i * 2 + k) * NEUR + m * P:
                                       _W_H0 + (i * 2 + k) * NEUR + (m + 1) * P],
                                    h[k][:, j * MM:(j + 1) * MM],
                                    start=(k == 0), stop=(k == 1))
                        nc.scalar.activation(h2[m][:], ps[:], AFT.Tanh,
                                             bias=bias(b_h0 + i * 2 + m))
                    h = h2
                # output layer: NEUR -> fout, linear (+bias on DVE)
                pso = psop.tile([fout, SC], F32, tag="pso")
                for j in range(SC // MM):
                    for k in range(2):
                        nc.tensor.matmul(
                            pso[:, j * MM:(j + 1) * MM],
                            ws[:, w_out0 + k * fout: w_out0 + (k + 1) * fout],
                            h[k][:, j * MM:(j + 1) * MM],
                            start=(k == 0), stop=(k == 1))
                o = opool.tile([fout, SC], F32, tag="o")
                nc.vector.tensor_scalar_add(o[:], pso[:], bias(b_out0, fout))
                nc.sync.dma_start(yt_d[:, c * SC:(c + 1) * SC], o[:])
    nc.compile()
    return nc


_MODULE_CACHE = {}


def _get_module(key, builder):
    if key not in _MODULE_CACHE:
        _MODULE_CACHE[key] = builder()
    return _MODULE_CACHE[key]


def _run_spmd(nc, in_maps):
    from concourse import bass_utils
    res = bass_utils.run_bass_kernel_spmd(nc, in_maps,
                                          core_ids=list(range(N_CORES)))
    return res.results


def kernel(x, enc_W_in, enc_b_in, enc_W_h, enc_b_h, enc_W_out, enc_b_out,
           dec_W_in, dec_b_in, dec_W_h, dec_b_h, dec_W_out, dec_b_out):
    x = np.ascontiguousarray(np.asarray(x, np.float32))

    enc_pack = _pack_weights(np.asarray(enc_W_in), np.asarray(enc_b_in),
                             np.asarray(enc_W_h), np.asarray(enc_b_h),
                             np.asarray(enc_W_out), np.asarray(enc_b_out))
    dec_pack = _pack_weights(np.asarray(dec_W_in), np.asarray(dec_b_in),
                             np.asarray(dec_W_h), np.asarray(dec_b_h),
                             np.asarray(dec_W_out), np.asarray(dec_b_out))

    enc_nc = _get_module("enc", lambda: _build_mlp_module(PHYS, LAT, TOK_ENC, False))
    dec_nc = _get_module("dec", lambda: _build_mlp_module(LAT, PHYS, TOK_DEC, True))

    # ---- launch 1: encoder ----
    in_maps = []
    for c in range(N_CORES):
        xs = x[c * BPC:(c + 1) * BPC].reshape(TOK_ENC, PHYS)
        in_maps.append({"xt": np.ascontiguousarray(xs.T), "wpack": enc_pack})
    enc_res = _run_spmd(enc_nc, in_maps)
    y = np.concatenate(
        [enc_res[c]["yt"].T.reshape(BPC, T, LAT) for c in range(N_CORES)], axis=0)

    # ---- host: EDMD chain, replicated verbatim from the reference ----
    import jax
    import jax.numpy as jnp
    cpu = jax.devices("cpu")[0]
    with jax.default_device(cpu):
        yj = jnp.asarray(y)
        Y = jnp.swapaxes(yj, 1, 2)
        Y_m = Y[:, :, :-1]
        Y_p = Y[:, :, 1:]
        U, sig, Vh = jnp.linalg.svd(Y_m, full_matrices=False)
        A = ((Y_p @ jnp.swapaxes(Vh, -1, -2)) * (1.0 / sig)[:, None, :]) \
            @ jnp.swapaxes(U, -1, -2)
        evals, evecs = jnp.linalg.eig(A)
        phi = jnp.linalg.solve(evecs, Y_m.astype(evecs.dtype))
        y0 = phi[:, :, 0]
        powers = evals[:, None, :] ** jnp.arange(P_STEPS)[None, :, None]
        y_adv = jnp.real(jnp.einsum('blm,bkm->bkl', evecs, powers * y0[:, None, :]))
        evals = np.asarray(evals)
        evecs = np.asarray(evecs)
        phi = np.asarray(phi)
        y_adv = np.asarray(y_adv)

    # ---- launch 2: decoder on [y ; y_adv] ----
    in_maps = []
    for c in range(N_CORES):
        z = np.empty((TOK_DEC, LAT), np.float32)
        z[:TOK_ENC] = y[c * BPC:(c + 1) * BPC].reshape(TOK_ENC, LAT)
        z[TOK_ENC:] = y_adv[c * BPC:(c + 1) * BPC].reshape(TOK_ENC, LAT)
        in_maps.append({"xt": np.ascontiguousarray(z.T), "wpack": dec_pack})
    dec_res = _run_spmd(dec_nc, in_maps)
    x_ae = np.concatenate(
        [dec_res[c]["yt"][:, :TOK_ENC].T.reshape(BPC, T, PHYS)
         for c in range(N_CORES)], axis=0)
    x_adv = np.concatenate(
        [dec_res[c]["yt"][:, TOK_ENC:].T.reshape(BPC, P_STEPS, PHYS)
         for c in range(N_CORES)], axis=0)

    return (y, x_ae, x_adv, y_adv, evals, evecs, phi)


# revision 4
# speedup vs baseline: 1.1436x; 1.1436x over previous
"""DLDMD kernel for Trainium2 (8 NeuronCores, batch-sharded).

Device (Bass/Tile, SPMD over 8 cores, 64 trajectories each):
  - encoder MLP   x  [B,T,3]  -> y     [B,T,32]   (fp32 matmuls, exact)
  - decoder MLP   y          -> x_ae   [B,T,3]    (float32r matmuls)
  - decoder MLP   y_adv      -> x_adv  [B,P,3]    (float32r matmuls)
Host (jnp on CPU, replicating the reference's EDMD chain op-for-op):
  - SVD -> A -> eig -> phi -> Vandermonde powers -> y_adv
  (complex nonsymmetric eig has no Trainium implementation; the chain is
  numerically chaotic so it must be replicated with the identical LAPACK
  calls the reference uses, seeded by the device-computed y.)
"""

import numpy as np

B, T, P_STEPS = 512, 256, 256
PHYS, LAT, NEUR, NLAYERS = 3, 32, 256, 4
N_CORES = 8
BPC = B // N_CORES              # trajectories per core
TOK_ENC = BPC * T               # encoder tokens per core
TOK_DEC = 2 * TOK_ENC           # decoder tokens per core (y ++ y_adv)
SC = 1024                       # tokens per superchunk (2 psum banks per tile)
MM = 512                        # moving-operand free size per matmul (fp32 max)
P = 128

# weight-pack column layout (one [128, WCOLS] fp32 array, single DMA)
_W_IN0 = 0                       # w_in   rows 0:Fin        cols [0, NEUR)
_W_H0 = NEUR                     # w_h    (i,k) -> NEUR cols each
_W_OUT0 = _W_H0 + NLAYERS * 2 * NEUR
def _pack_cols(fout):
    w_out0 = _W_OUT0
    b_in0 = w_out0 + 2 * fout
    b_h0 = b_in0 + 2
    b_out0 = b_h0 + NLAYERS * 2
    return w_out0, b_in0, b_h0, b_out0, b_out0 + 1


def _pack_weights(w_in, b_in, w_h, b_h, w_out, b_out):
    fin, fout = w_in.shape[0], w_out.shape[1]
    w_out0, b_in0, b_h0, b_out0, wcols = _pack_cols(fout)
    pk = np.zeros((P, wcols), np.float32)
    pk[0:fin, _W_IN0:_W_IN0 + NEUR] = w_in
    for i in range(NLAYERS):
        for k in range(2):
            pk[:, _W_H0 + (i * 2 + k) * NEUR: _W_H0 + (i * 2 + k + 1) * NEUR] = \
                w_h[i, k * P:(k + 1) * P, :]
    for k in range(2):
        pk[:, w_out0 + k * fout: w_out0 + (k + 1) * fout] = w_out[k * P:(k + 1) * P, :]
    pk[:, b_in0] = b_in[0:P]
    pk[:, b_in0 + 1] = b_in[P:NEUR]
    for i in range(NLAYERS):
        for m in range(2):
            pk[:, b_h0 + i * 2 + m] = b_h[i, m * P:(m + 1) * P]
    pk[0:fout, b_out0] = b_out
    return pk


def _build_mlp_module(fin, fout, ntok, use_f32r):
    """One SPMD module: xt [fin, ntok] -> yt [fout, ntok] through the MLP."""
    import concourse.bacc as bacc
    import concourse.tile as tile
    import concourse.mybir as mybir

    F32 = mybir.dt.float32
    MMDT = mybir.dt.float32r if use_f32r else F32
    AFT = mybir.ActivationFunctionType

    w_out0, b_in0, b_h0, b_out0, wcols = _pack_cols(fout)

    nc = bacc.Bacc("TRN2", target_bir_lowering=False, debug=False,
                   num_devices=N_CORES)
    xt_d = nc.dram_tensor("xt", [fin, ntok], MMDT, kind="ExternalInput").ap()
    w_d = nc.dram_tensor("wpack", [P, wcols], MMDT, kind="ExternalInput").ap()
    yt_d = nc.dram_tensor("yt", [fout, ntok], F32, kind="ExternalOutput").ap()

    nsc = ntok // SC
    GRP = 2  # superchunks interleaved per emission wave
    with tile.TileContext(nc) as tc:
        with tc.tile_pool(name="wp", bufs=1) as wp, \
             tc.tile_pool(name="ap", bufs=4) as apool, \
             tc.tile_pool(name="hp", bufs=12) as hpool, \
             tc.tile_pool(name="op", bufs=6) as opool, \
             tc.tile_pool(name="ps", bufs=3, space="PSUM") as psp, \
             tc.tile_pool(name="pso", bufs=2, space="PSUM") as psop:
            ws = wp.tile([P, wcols], MMDT)
            nc.sync.dma_start(ws[:], w_d[:, :])
            # dummy matmul: absorbs the weight-DMA wait so every real matmul
            # needs at most one sync wait (LDWEIGHTS allows only one).
            dps = psop.tile([1, 1], F32, tag="pso", name="dummy_ps")
            nc.tensor.matmul(dps[0:1, 0:1], ws[:, 0:1].bitcast(F32),
                             ws[:, 1:2].bitcast(F32), start=True, stop=True)

            def bias(col, rows=P):
                return ws[0:rows, col:col + 1].bitcast(F32)

            def in_layer(c, a):
                h = [hpool.tile([P, SC], MMDT, tag="h", name=f"h{c}_in{m}")
                     for m in range(2)]
                for m in range(2):
                    ps = psp.tile([P, SC], F32, tag="ps", name=f"ps{c}_in{m}")
                    for j in range(SC // MM):
                        nc.tensor.matmul(
                            ps[:, j * MM:(j + 1) * MM],
                            ws[0:fin, _W_IN0 + m * P: _W_IN0 + (m + 1) * P],
                            a[:, j * MM:(j + 1) * MM], start=True, stop=True)
                    nc.scalar.activation(h[m][:], ps[:], AFT.Tanh,
                                         bias=bias(b_in0 + m))
                return h

            def hidden_layer(c, i, h):
                h2 = [hpool.tile([P, SC], MMDT, tag="h", name=f"h{c}_l{i}_{m}")
                      for m in range(2)]
                for m in range(2):
                    ps = psp.tile([P, SC], F32, tag="ps", name=f"ps{c}_l{i}_{m}")
                    for j in range(SC // MM):
                        for k in range(2):
                            nc.tensor.matmul(
                                ps[:, j * MM:(j + 1) * MM],
                                ws[:, _W_H0 + (i * 2 + k) * NEUR + m * P:
                                   _W_H0 + (i * 2 + k) * NEUR + (m + 1) * P],
                                h[k][:, j * MM:(j + 1) * MM],
                                start=(k == 0), stop=(k == 1))
                    nc.scalar.activation(h2[m][:], ps[:], AFT.Tanh,
                                         bias=bias(b_h0 + i * 2 + m))
                return h2

            def out_layer(c, h):
                for j in range(SC // MM):
                    pso = psop.tile([fout, MM], F32, tag="pso", name=f"pso{c}_{j}")
                    for k in range(2):
                        nc.tensor.matmul(
                            pso[:],
                            ws[:, w_out0 + k * fout: w_out0 + (k + 1) * fout],
                            h[k][:, j * MM:(j + 1) * MM],
                            start=(k == 0), stop=(k == 1))
                    o = opool.tile([fout, MM], F32, tag="o", name=f"o{c}_{j}")
                    nc.vector.tensor_scalar_add(o[:], pso[:], bias(b_out0, fout))
                    nc.sync.dma_start(
                        yt_d[:, c * SC + j * MM: c * SC + (j + 1) * MM], o[:])

            for c0 in range(0, nsc, GRP):
                subs = range(c0, min(c0 + GRP, nsc))
                avs = []
                for c in subs:
                    a = apool.tile([fin, SC], MMDT, tag="a", name=f"a{c}")
                    nc.sync.dma_start(a[:], xt_d[:, c * SC:(c + 1) * SC])
                    avs.append(a)
                hs = [in_layer(c, a) for c, a in zip(subs, avs)]
                for i in range(NLAYERS):
                    hs = [hidden_layer(c, i, h) for c, h in zip(subs, hs)]
                for c, h in zip(subs, hs):
                    out_layer(c, h)
    nc.compile()
    return nc


_MODULE_CACHE = {}


def _get_module(key, builder):
    if key not in _MODULE_CACHE:
        _MODULE_CACHE[key] = builder()
    return _MODULE_CACHE[key]


def _run_spmd(nc, in_maps):
    from concourse import bass_utils
    res = bass_utils.run_bass_kernel_spmd(nc, in_maps,
                                          core_ids=list(range(N_CORES)))
    return res.results


def kernel(x, enc_W_in, enc_b_in, enc_W_h, enc_b_h, enc_W_out, enc_b_out,
           dec_W_in, dec_b_in, dec_W_h, dec_b_h, dec_W_out, dec_b_out):
    x = np.ascontiguousarray(np.asarray(x, np.float32))

    enc_pack = _pack_weights(np.asarray(enc_W_in), np.asarray(enc_b_in),
                             np.asarray(enc_W_h), np.asarray(enc_b_h),
                             np.asarray(enc_W_out), np.asarray(enc_b_out))
    dec_pack = _pack_weights(np.asarray(dec_W_in), np.asarray(dec_b_in),
                             np.asarray(dec_W_h), np.asarray(dec_b_h),
                             np.asarray(dec_W_out), np.asarray(dec_b_out))

    enc_nc = _get_module("enc", lambda: _build_mlp_module(PHYS, LAT, TOK_ENC, False))
    dec_nc = _get_module("dec", lambda: _build_mlp_module(LAT, PHYS, TOK_DEC, True))

    # ---- launch 1: encoder ----
    in_maps = []
    for c in range(N_CORES):
        xs = x[c * BPC:(c + 1) * BPC].reshape(TOK_ENC, PHYS)
        in_maps.append({"xt": np.ascontiguousarray(xs.T), "wpack": enc_pack})
    enc_res = _run_spmd(enc_nc, in_maps)
    y = np.concatenate(
        [enc_res[c]["yt"].T.reshape(BPC, T, LAT) for c in range(N_CORES)], axis=0)

    # ---- host: EDMD chain, replicated verbatim from the reference ----
    import jax
    import jax.numpy as jnp
    cpu = jax.devices("cpu")[0]
    with jax.default_device(cpu):
        yj = jnp.asarray(y)
        Y = jnp.swapaxes(yj, 1, 2)
        Y_m = Y[:, :, :-1]
        Y_p = Y[:, :, 1:]
        U, sig, Vh = jnp.linalg.svd(Y_m, full_matrices=False)
        A = ((Y_p @ jnp.swapaxes(Vh, -1, -2)) * (1.0 / sig)[:, None, :]) \
            @ jnp.swapaxes(U, -1, -2)
        evals, evecs = jnp.linalg.eig(A)
        phi = jnp.linalg.solve(evecs, Y_m.astype(evecs.dtype))
        y0 = phi[:, :, 0]
        powers = evals[:, None, :] ** jnp.arange(P_STEPS)[None, :, None]
        y_adv = jnp.real(jnp.einsum('blm,bkm->bkl', evecs, powers * y0[:, None, :]))
        evals = np.asarray(evals)
        evecs = np.asarray(evecs)
        phi = np.asarray(phi)
        y_adv = np.asarray(y_adv)

    # ---- launch 2: decoder on [y ; y_adv] ----
    in_maps = []
    for c in range(N_CORES):
        z = np.empty((TOK_DEC, LAT), np.float32)
        z[:TOK_ENC] = y[c * BPC:(c + 1) * BPC].reshape(TOK_ENC, LAT)
        z[TOK_ENC:] = y_adv[c * BPC:(c + 1) * BPC].reshape(TOK_ENC, LAT)
        in_maps.append({"xt": np.ascontiguousarray(z.T), "wpack": dec_pack})
    dec_res = _run_spmd(dec_nc, in_maps)
    x_ae = np.concatenate(
        [dec_res[c]["yt"][:, :TOK_ENC].T.reshape(BPC, T, PHYS)
         for c in range(N_CORES)], axis=0)
    x_adv = np.concatenate(
        [dec_res[c]["yt"][:, TOK_ENC:].T.reshape(BPC, P_STEPS, PHYS)
         for c in range(N_CORES)], axis=0)

    return (y, x_ae, x_adv, y_adv, evals, evecs, phi)
